# revision 10
# baseline (speedup 1.0000x reference)
"""DSNT + leaky-integrator kernel for Trainium2 (8 NeuronCores, SPMD).

Math (matches the reference):
  px[w] = linspace(-1, 1, W);  py[h] = linspace(-1, 1, H)
  co_1[t] = sum_{h,w} x[t,h,w] * px[w]      (expected x-coordinate)
  co_2[t] = sum_{h,w} x[t,h,w] * py[h]      (expected y-coordinate)
  cos[t]  = (co_2[t], co_1[t])
  LI scan over t:  s = s - s*li_tm + cos[t]  ->  out[t] = s

Strategy:
  - Shard T=512 across 8 cores (64 frames each, ~75 MB per core); the
    DSNT reduction is embarrassingly parallel and memory-bound.
  - Per core, per frame: one 1.2 MB DMA loads [480, 640] into a
    [120, 4, 640] SBUF tile with h = 4p + j, so each partition receives
    one contiguous 10 KB run from DRAM (max DMA descriptor efficiency).
  - TensorE contracts the h (partition) dim: per j-subset, a [120, 2]
    stationary matrix (col0 = py[4p+j], col1 = ones) against the moving
    [120, 320] x slices, accumulating PSUM [2, 320] x2 over the 4
    subsets.  Row 0 = sum_h py*x, row 1 = sum_h x (per w).
  - VectorE multiplies PSUM by [ones; px] and reduces along w ->
    cos pairs, collected in a [2, 64] tile, DMA'd out once.
  - The 2-element LI recurrence is O(T) scalar work, done on host on the
    gathered [512, 2] cos values.
"""

import numpy as np
from contextlib import ExitStack

import concourse.bass as bass
import concourse.bacc as bacc
import concourse.tile as tile
from concourse import mybir
from concourse.bass_utils import run_bass_kernel_spmd

N_CORES = 8
T, H, W = 512, 480, 640
TL = T // N_CORES            # 64 frames per core
P = 120                      # partition rows per h-chunk
C = H // P                   # 4 h-chunks
WH = W // 2                  # 320, one PSUM bank per half
F32 = mybir.dt.float32

# Config knobs (test harness may override before first kernel() call).
USE_F32R = False             # run the matmuls in float32r (4x PE throughput)
X_BUFS = 6
TRACE = False
TRACE_CORES = None
LAST_RESULT = None

_NC_CACHE = {}


def _build_nc(use_f32r):
    nc = bacc.Bacc(
        "TRN2", target_bir_lowering=False, debug=False, num_devices=N_CORES
    )
    x = nc.dram_tensor("x", [TL, H, W], F32, kind="ExternalInput")
    res = nc.dram_tensor("res", [2, TL], F32, kind="ExternalOutput")

    px = np.linspace(-1.0, 1.0, W).astype(np.float32)           # [W]
    py = np.linspace(-1.0, 1.0, H).astype(np.float32)           # [H]
    wts_np = np.stack([py, np.ones(H, np.float32)], axis=1)     # [H, 2]
    # h = 4p + j: stationary weights for j-subset j are rows j::4.
    wts_r = np.stack([wts_np[j::C, :] for j in range(C)])       # [C, P, 2]
    coeff_np = np.stack([np.ones(W, np.float32), px], axis=0)   # [2, W]
    wts_d = nc.inline_tensor(np.ascontiguousarray(wts_r), name="wts_const")
    coeff_d = nc.inline_tensor(np.ascontiguousarray(coeff_np), name="coeff_const")

    mmdt = mybir.dt.float32r if use_f32r else F32

    with tile.TileContext(nc) as tc, ExitStack() as ctx:
        consts = ctx.enter_context(tc.tile_pool(name="consts", bufs=1))
        xpool = ctx.enter_context(tc.tile_pool(name="xtiles", bufs=X_BUFS))
        scr = ctx.enter_context(tc.tile_pool(name="scratch", bufs=3))
        psum = ctx.enter_context(tc.tile_pool(name="psum", bufs=4, space="PSUM"))
        outp = ctx.enter_context(tc.tile_pool(name="outp", bufs=1))

        lhsT = [consts.tile([P, 2], F32, name=f"lhsT{c}", tag=f"lhsT{c}")
                for c in range(C)]
        for c in range(C):
            nc.gpsimd.dma_start(out=lhsT[c], in_=wts_d[c])
        coeff = consts.tile([2, W], F32, name="coeff")
        nc.gpsimd.dma_start(out=coeff, in_=coeff_d[:])

        resbuf = outp.tile([2, TL], F32, name="resbuf")

        for f in range(TL):
            xt = xpool.tile([P, C, W], F32, tag="xt", name="xt")
            nc.sync.dma_start(
                out=xt, in_=x[f].rearrange("(p j) w -> p j w", j=C)
            )
            pf = [psum.tile([2, WH], F32, tag=f"pf{h}", name=f"pf{h}")
                  for h in range(2)]
            for hw in range(2):
                sl = slice(hw * WH, (hw + 1) * WH)
                for c in range(C):
                    nc.tensor.matmul(
                        pf[hw],
                        lhsT[c][:].bitcast(mmdt),
                        xt[:, c, sl].bitcast(mmdt),
                        start=(c == 0),
                        stop=(c == C - 1),
                    )
            tmp = scr.tile([2, W], F32, tag="tmp", name="tmp")
            for hw in range(2):
                sl = slice(hw * WH, (hw + 1) * WH)
                nc.vector.tensor_mul(tmp[:, sl], pf[hw], coeff[:, sl])
            nc.vector.tensor_reduce(
                out=resbuf[:, f : f + 1],
                in_=tmp,
                axis=mybir.AxisListType.X,
                op=mybir.AluOpType.add,
            )

        nc.sync.dma_start(out=res[:], in_=resbuf)

    nc.finalize()
    return nc


def _get_nc():
    key = ("f32r" if USE_F32R else "f32", X_BUFS)
    if key not in _NC_CACHE:
        _NC_CACHE[key] = _build_nc(USE_F32R)
    return _NC_CACHE[key]


def kernel(x, li_tm, state):
    global LAST_RESULT
    x = np.ascontiguousarray(np.asarray(x, dtype=np.float32))
    li_tm = np.asarray(li_tm, dtype=np.float32)
    state = np.asarray(state, dtype=np.float32)
    assert x.shape == (T, H, W)

    nc = _get_nc()
    in_maps = [{"x": x[i * TL : (i + 1) * TL]} for i in range(N_CORES)]
    kwargs = {}
    if TRACE:
        kwargs["trace"] = True
        if TRACE_CORES is not None:
            kwargs["trace_cores"] = list(TRACE_CORES)
    r = run_bass_kernel_spmd(nc, in_maps, list(range(N_CORES)), **kwargs)
    LAST_RESULT = r
    # res[i] is [2, TL]: row 0 = co_2, row 1 = co_1 -> cos [T, 2]
    cos = np.concatenate(
        [r.results[i]["res"].T for i in range(N_CORES)], axis=0
    ).astype(np.float32)

    # Leaky integrator (tiny 2-element recurrence), fp32 like the reference.
    s = state.copy()
    out = np.empty((T, 2), dtype=np.float32)
    for t in range(T):
        s = s - s * li_tm + cos[t]
        out[t] = s
    return out, out[-1].copy()


# revision 11
# speedup vs baseline: 1.3871x; 1.3871x over previous
"""DSNT + leaky-integrator kernel for Trainium2 (8 NeuronCores, SPMD).

Math (matches the reference):
  px[w] = linspace(-1, 1, W);  py[h] = linspace(-1, 1, H)
  co_1[t] = sum_{h,w} x[t,h,w] * px[w]      (expected x-coordinate)
  co_2[t] = sum_{h,w} x[t,h,w] * py[h]      (expected y-coordinate)
  cos[t]  = (co_2[t], co_1[t])
  LI scan over t:  s = s - s*li_tm + cos[t]  ->  out[t] = s

Strategy:
  - Shard T=512 across 8 cores (64 frames each, ~75 MB per core); the
    DSNT reduction is embarrassingly parallel and memory-bound.
  - Per core, frames are processed in groups of 4 (1920 rows = 128
    partitions x 15 rows), so every DMA uses all 128 partitions with one
    contiguous 38.4 KB descriptor per partition -- measured ~2x DMA
    bandwidth vs any <128-partition layout on this part.
  - TensorE contracts the partition dim: for each of the 15 row-slots j,
    a [128, 8] stationary matrix (per frame g: col 2g = py at that row,
    col 2g+1 = 1, zero outside frame g's partition range) against moving
    [128, 320] x slices, accumulating PSUM [8, 320] x2 over j.
    Row 2g = sum_h py*x, row 2g+1 = sum_h x  (per w, frame g).
  - VectorE multiplies PSUM by [ones; px] rows and reduces along w ->
    (co_2, co_1) pairs, collected in an [8, 16] tile, DMA'd out once.
  - The 2-element LI recurrence is O(T) scalar work, done on host on the
    gathered [512, 2] cos values.
"""

import numpy as np
from contextlib import ExitStack

import concourse.bass as bass
import concourse.bacc as bacc
import concourse.tile as tile
from concourse import mybir
from concourse.bass_utils import run_bass_kernel_spmd

N_CORES = 8
T, H, W = 512, 480, 640
TL = T // N_CORES            # 64 frames per core
FG = 4                       # frames per group
NG = TL // FG                # 16 groups per core
RP = FG * H // 128           # 15 rows per partition
PF = 128 // FG               # 32 partitions per frame
WH = W // 2                  # 320, one PSUM bank per half
F32 = mybir.dt.float32

# Config knobs (test harness may override before first kernel() call).
USE_F32R = False             # run the matmuls in float32r (4x PE throughput)
X_BUFS = 3
TRACE = False
TRACE_CORES = None
LAST_RESULT = None

_NC_CACHE = {}


def _build_nc(use_f32r):
    nc = bacc.Bacc(
        "TRN2", target_bir_lowering=False, debug=False, num_devices=N_CORES
    )
    x = nc.dram_tensor("x", [TL * H * W], F32, kind="ExternalInput")
    res = nc.dram_tensor("res", [2 * FG, NG], F32, kind="ExternalOutput")

    px = np.linspace(-1.0, 1.0, W).astype(np.float32)           # [W]
    py = np.linspace(-1.0, 1.0, H).astype(np.float32)           # [H]
    # Stationary weights: [p, j, m] with m = 2*FG columns.
    wts_np = np.zeros((128, RP, 2 * FG), dtype=np.float32)
    for p in range(128):
        g, a = divmod(p, PF)
        for j in range(RP):
            wts_np[p, j, 2 * g] = py[a * RP + j]
            wts_np[p, j, 2 * g + 1] = 1.0
    # PSUM evacuation coefficients: row 2g -> ones (co_2), 2g+1 -> px (co_1)
    coeff_np = np.tile(
        np.stack([np.ones(W, np.float32), px], axis=0), (FG, 1)
    )                                                            # [8, W]
    wts_d = nc.inline_tensor(np.ascontiguousarray(wts_np), name="wts_const")
    coeff_d = nc.inline_tensor(np.ascontiguousarray(coeff_np), name="coeff_const")

    mmdt = mybir.dt.float32r if use_f32r else F32
    GE = FG * H * W                                              # elems per group

    with tile.TileContext(nc) as tc, ExitStack() as ctx:
        consts = ctx.enter_context(tc.tile_pool(name="consts", bufs=1))
        xpool = ctx.enter_context(tc.tile_pool(name="xtiles", bufs=X_BUFS))
        scr = ctx.enter_context(tc.tile_pool(name="scratch", bufs=3))
        psum = ctx.enter_context(tc.tile_pool(name="psum", bufs=4, space="PSUM"))
        outp = ctx.enter_context(tc.tile_pool(name="outp", bufs=1))

        lhsT = consts.tile([128, RP, 2 * FG], F32, name="lhsT")
        nc.gpsimd.dma_start(out=lhsT, in_=wts_d[:])
        coeff = consts.tile([2 * FG, W], F32, name="coeff")
        nc.gpsimd.dma_start(out=coeff, in_=coeff_d[:])

        resbuf = outp.tile([2 * FG, NG], F32, name="resbuf")

        for g in range(NG):
            xt = xpool.tile([128, RP, W], F32, tag="xt", name="xt")
            nc.sync.dma_start(
                out=xt,
                in_=x[g * GE : (g + 1) * GE].rearrange("(p e) -> p e", p=128),
            )
            pf = [psum.tile([2 * FG, WH], F32, tag=f"pf{h}", name=f"pf{h}")
                  for h in range(2)]
            for hw in range(2):
                sl = slice(hw * WH, (hw + 1) * WH)
                for j in range(RP):
                    nc.tensor.matmul(
                        pf[hw],
                        lhsT[:, j, :].bitcast(mmdt),
                        xt[:, j, sl].bitcast(mmdt),
                        start=(j == 0),
                        stop=(j == RP - 1),
                    )
            tmp = scr.tile([2 * FG, W], F32, tag="tmp", name="tmp")
            for hw in range(2):
                sl = slice(hw * WH, (hw + 1) * WH)
                nc.vector.tensor_mul(tmp[:, sl], pf[hw], coeff[:, sl])
            nc.vector.tensor_reduce(
                out=resbuf[:, g : g + 1],
                in_=tmp,
                axis=mybir.AxisListType.X,
                op=mybir.AluOpType.add,
            )

        nc.sync.dma_start(out=res[:], in_=resbuf)

    nc.finalize()
    return nc


def _get_nc():
    key = ("f32r" if USE_F32R else "f32", X_BUFS)
    if key not in _NC_CACHE:
        _NC_CACHE[key] = _build_nc(USE_F32R)
    return _NC_CACHE[key]


def kernel(x, li_tm, state):
    global LAST_RESULT
    x = np.ascontiguousarray(np.asarray(x, dtype=np.float32))
    li_tm = np.asarray(li_tm, dtype=np.float32)
    state = np.asarray(state, dtype=np.float32)
    assert x.shape == (T, H, W)

    nc = _get_nc()
    xf = x.reshape(N_CORES, TL * H * W)
    in_maps = [{"x": xf[i]} for i in range(N_CORES)]
    kwargs = {}
    if TRACE:
        kwargs["trace"] = True
        if TRACE_CORES is not None:
            kwargs["trace_cores"] = list(TRACE_CORES)
    r = run_bass_kernel_spmd(nc, in_maps, list(range(N_CORES)), **kwargs)
    LAST_RESULT = r
    # res[i] is [8, NG]: rows (2g, 2g+1) = (co_2, co_1) of frame gidx*FG+g
    cos = np.concatenate(
        [
            r.results[i]["res"].reshape(FG, 2, NG).transpose(2, 0, 1).reshape(TL, 2)
            for i in range(N_CORES)
        ],
        axis=0,
    ).astype(np.float32)

    # Leaky integrator (tiny 2-element recurrence), fp32 like the reference.
    s = state.copy()
    out = np.empty((T, 2), dtype=np.float32)
    for t in range(T):
        s = s - s * li_tm + cos[t]
        out[t] = s
    return out, out[-1].copy()


# revision 16
# speedup vs baseline: 1.9256x; 1.3881x over previous
"""DSNT + leaky-integrator kernel for Trainium2 (8 NeuronCores, SPMD).

Math (matches the reference):
  px[w] = linspace(-1, 1, W);  py[h] = linspace(-1, 1, H)
  co_1[t] = sum_{h,w} x[t,h,w] * px[w]      (expected x-coordinate)
  co_2[t] = sum_{h,w} x[t,h,w] * py[h]      (expected y-coordinate)
  cos[t]  = (co_2[t], co_1[t])
  LI scan over t:  s = s - s*li_tm + cos[t]  ->  out[t] = s

Strategy:
  - Shard T=512 across 8 cores (64 frames each, ~75 MB per core); the
    DSNT reduction is embarrassingly parallel and memory-bound.
  - Per core, frames are processed in groups of 4 (1920 rows = 128
    partitions x 15 rows), so every DMA uses all 128 partitions with one
    contiguous 38.4 KB descriptor per partition -- measured ~2x DMA
    bandwidth vs any <128-partition layout on this part.
  - TensorE contracts the partition dim: for each of the 15 row-slots j,
    a [128, 8] stationary matrix (per frame g: col 2g = py at that row,
    col 2g+1 = 1, zero outside frame g's partition range) against moving
    [128, 320] x slices, accumulating PSUM [8, 320] x2 over j.
    Row 2g = sum_h py*x, row 2g+1 = sum_h x  (per w, frame g).
  - VectorE multiplies PSUM by [ones; px] rows and reduces along w ->
    (co_2, co_1) pairs, collected in an [8, 16] tile, DMA'd out once.
  - The 2-element LI recurrence is O(T) scalar work, done on host on the
    gathered [512, 2] cos values.
"""

import numpy as np
from contextlib import ExitStack

import concourse.bass as bass
import concourse.bacc as bacc
import concourse.tile as tile
from concourse import mybir
from concourse.bass_utils import run_bass_kernel_spmd

N_CORES = 8
T, H, W = 512, 480, 640
TL = T // N_CORES            # 64 frames per core
FG = 4                       # frames per group
NG = TL // FG                # 16 groups per core
RP = FG * H // 128           # 15 rows per partition
PF = 128 // FG               # 32 partitions per frame
WH = W // 2                  # 320, one PSUM bank per half
F32 = mybir.dt.float32

# Config knobs (test harness may override before first kernel() call).
USE_F32R = False             # run the matmuls in float32r (4x PE throughput)
X_BUFS = 3
TRACE = False
TRACE_CORES = None
LAST_RESULT = None

_NC_CACHE = {}


def _build_nc(use_f32r):
    nc = bacc.Bacc(
        "TRN2", target_bir_lowering=False, debug=False, num_devices=N_CORES
    )
    mmdt = mybir.dt.float32r if use_f32r else F32
    x = nc.dram_tensor("x", [TL * H * W], mmdt, kind="ExternalInput")
    res = nc.dram_tensor("res", [2 * FG, NG], F32, kind="ExternalOutput")

    px = np.linspace(-1.0, 1.0, W).astype(np.float32)           # [W]
    py = np.linspace(-1.0, 1.0, H).astype(np.float32)           # [H]
    # Stationary weights: [p, j, m] with m = 2*FG columns.
    wts_np = np.zeros((128, RP, 2 * FG), dtype=np.float32)
    for p in range(128):
        g, a = divmod(p, PF)
        for j in range(RP):
            wts_np[p, j, 2 * g] = py[a * RP + j]
            wts_np[p, j, 2 * g + 1] = 1.0
    # PSUM evacuation coefficients: row 2g -> ones (co_2), 2g+1 -> px (co_1)
    coeff_np = np.tile(
        np.stack([np.ones(W, np.float32), px], axis=0), (FG, 1)
    )                                                            # [8, W]
    wts_d = nc.inline_tensor(np.ascontiguousarray(wts_np), name="wts_const")
    coeff_d = nc.inline_tensor(np.ascontiguousarray(coeff_np), name="coeff_const")

    GE = FG * H * W                                              # elems per group

    with tile.TileContext(nc) as tc, ExitStack() as ctx:
        consts = ctx.enter_context(tc.tile_pool(name="consts", bufs=1))
        xpool = ctx.enter_context(tc.tile_pool(name="xtiles", bufs=X_BUFS))
        scr = ctx.enter_context(tc.tile_pool(name="scratch", bufs=3))
        psum = ctx.enter_context(tc.tile_pool(name="psum", bufs=4, space="PSUM"))
        outp = ctx.enter_context(tc.tile_pool(name="outp", bufs=1))

        lhsT_f32 = consts.tile([128, RP, 2 * FG], F32, name="lhsT_f32")
        nc.gpsimd.dma_start(out=lhsT_f32, in_=wts_d[:])
        if use_f32r:
            lhsT = consts.tile([128, RP, 2 * FG], mmdt, name="lhsT")
            nc.vector.tensor_copy(lhsT, lhsT_f32)
        else:
            lhsT = lhsT_f32
        coeff = consts.tile([2 * FG, W], F32, name="coeff")
        nc.gpsimd.dma_start(out=coeff, in_=coeff_d[:])

        resbuf = outp.tile([2 * FG, NG], F32, name="resbuf")

        for g in range(NG):
            xt = xpool.tile([128, RP, W], mmdt, tag="xt", name="xt")
            nc.sync.dma_start(
                out=xt,
                in_=x[g * GE : (g + 1) * GE].rearrange("(p e) -> p e", p=128),
            )
            pf = [psum.tile([2 * FG, WH], F32, tag=f"pf{h}", name=f"pf{h}")
                  for h in range(2)]
            for hw in range(2):
                sl = slice(hw * WH, (hw + 1) * WH)
                for j in range(RP):
                    nc.tensor.matmul(
                        pf[hw],
                        lhsT[:, j, :],
                        xt[:, j, sl],
                        start=(j == 0),
                        stop=(j == RP - 1),
                    )
            tmp = scr.tile([2 * FG, W], F32, tag="tmp", name="tmp")
            for hw in range(2):
                sl = slice(hw * WH, (hw + 1) * WH)
                nc.vector.tensor_mul(tmp[:, sl], pf[hw], coeff[:, sl])
            nc.vector.tensor_reduce(
                out=resbuf[:, g : g + 1],
                in_=tmp,
                axis=mybir.AxisListType.X,
                op=mybir.AluOpType.add,
            )

        nc.sync.dma_start(out=res[:], in_=resbuf)

    nc.finalize()
    return nc


def _get_nc():
    key = ("f32r" if USE_F32R else "f32", X_BUFS)
    if key not in _NC_CACHE:
        _NC_CACHE[key] = _build_nc(USE_F32R)
    return _NC_CACHE[key]


def kernel(x, li_tm, state):
    global LAST_RESULT
    x = np.ascontiguousarray(np.asarray(x, dtype=np.float32))
    li_tm = np.asarray(li_tm, dtype=np.float32)
    state = np.asarray(state, dtype=np.float32)
    assert x.shape == (T, H, W)

    nc = _get_nc()
    xf = x.reshape(N_CORES, TL * H * W)
    in_maps = [{"x": xf[i]} for i in range(N_CORES)]
    kwargs = {}
    if TRACE:
        kwargs["trace"] = True
        if TRACE_CORES is not None:
            kwargs["trace_cores"] = list(TRACE_CORES)
    r = run_bass_kernel_spmd(nc, in_maps, list(range(N_CORES)), **kwargs)
    LAST_RESULT = r
    # res[i] is [8, NG]: rows (2g, 2g+1) = (co_2, co_1) of frame gidx*FG+g
    cos = np.concatenate(
        [
            r.results[i]["res"].reshape(FG, 2, NG).transpose(2, 0, 1).reshape(TL, 2)
            for i in range(N_CORES)
        ],
        axis=0,
    ).astype(np.float32)

    # Leaky integrator (tiny 2-element recurrence), fp32 like the reference.
    s = state.copy()
    out = np.empty((T, 2), dtype=np.float32)
    for t in range(T):
        s = s - s * li_tm + cos[t]
        out[t] = s
    return out, out[-1].copy()
